# revision 29
# baseline (speedup 1.0000x reference)
"""Trainium2 Bass kernel for deformable conv2d (nn_DeformByDepthConv2d).

Strategy: data-parallel over batch (8 images -> 8 NeuronCores). Per core:
  1. Build a bilinear "difference table" in DRAM: for every padded pixel
     position r=(y,x), row = [V, Dy, Dx, Dxy] (64 ch each, bf16, 512B):
       V   = rgb[:, y, x]
       Dy  = rgb[:, y+1, x] - rgb[:, y, x]
       Dx  = rgb[:, y, x+1] - rgb[:, y, x]
       Dxy = rgb[:, y+1, x+1] - rgb[:, y+1, x] - rgb[:, y, x+1] + rgb[:, y, x]
     The bilinear sample is then exactly: V + fy*Dy + fx*Dx + fx*fy*Dxy
     -- so ONE gathered row per (tap, pixel) fetches everything needed.
  2. Compute gather rows + poly weights (fy, fx, fx*fy) from `offsets`
     on-device (DVE). Two coordinate pipelines: pixel-major for wpoly,
     and the dma_gather wrapped-16 idx layout (int16) directly from
     host-rearranged (layout-only) coordinate inputs. y and x are
     processed interleaved (.., 2) to halve instruction count.
  3. Gather via batched ext-isa dma_gather (InstDMAGatherAnt): 36 calls
     of 1024 rows x 512B, round-robined over 4 SWDGE queues so the 4x16
     DMA rings drain in parallel (~150 GB/s vs 46 GB/s on one queue).
     Calls are paced by a bufs=8 tile ring (the DVE combine consuming a
     gather tile gates the call 8 slots later) -- this runtime has no
     ring-overflow reclaim, so outstanding descriptors must stay under
     ~2 calls/queue. num_idxs > 1024 per call crashes the runtime.
     Pool runs ONLY the gathers; canvas loads go first on the sync
     HWDGE queue and coordinate inputs on the scalar HWDGE queue so the
     first gather can start ~25us in (was 111us).
  4. DVE poly combine -> col[128px, 32, tap, 64ch] (bf16).
  5. PE transposes col -> colT[(tap,ch) 576, 4096px], then the conv is a
     [576,128]^T x [576,4096] matmul accumulated over 5 K-chunks in PSUM.
  6. ACT adds bias on PSUM->SBUF copy; DMA out fp32 [128, 4096].

Host side does layout-only prep (transposes/padding/replication of inputs);
all arithmetic (diffs, coords, weights, conv) runs on device.
"""

import sys

sys.path.insert(0, "/opt/trn_rl_repo")

from contextlib import ExitStack

import numpy as np

import concourse.bass as bass
import concourse.tile as tile
from concourse import bacc, mybir
from concourse.bass_utils import run_bass_kernel_spmd
from concourse.masks import make_identity
from concourse.tile import add_dep_helper

F32 = mybir.dt.float32
BF16 = mybir.dt.bfloat16
I16 = mybir.dt.int16
I32 = mybir.dt.int32

B, CIN, COUT, KH, KW = 8, 64, 128, 3, 3
H = W = 64
K = KH * KW            # 9 taps
P = H * W              # 4096 pixels
PAD = 8                # table padding on each side
HP = WP = H + 2 * PAD  # 80
NROW = HP * WP         # 6400 table rows
NG = P // 128          # 32 pixel groups of 128
ELEM = 4 * CIN         # table row: V|Dy|Dx|Dxy x 64ch = 256 bf16 = 512B
KC = K * CIN           # 576 contraction rows
NK = 5                 # K chunks: 4x128 + 1x64
NCH = 8                # output N chunks of 512

NQ = 4                 # SWDGE queues (max supported)
NR = 4                 # gather ranges
GRL = NG // NR         # pixel groups per range = 8
NIDX = GRL * 128       # rows per dma_gather call = 1024 (hard runtime max)
SLOT = NIDX // 16      # idx free slots per call = 64

NSL = 4                # table slices
RSL = 1664             # rows per slice; last = 1408
# per-range gather view (rows) and matching y-clamp hi (y+16 scale):
# range r only reads rows < VIEW[r] because its clamped y0 <= CLAMP_HI[r]
VIEW = [2560, 3840, 5120, 6400]
CLAMP_HI = [39.99, 55.99, 71.99, 86.99]
# table slices each range's gathers must wait for
SL_NEED = [2, 3, 4, 4]

TRACE = False
LAST_EXEC_NS = None
_PROG = None


def _build_program():
    nc = bacc.Bacc(
        "TRN2", target_bir_lowering=False, debug=False, num_devices=8,
        num_swdge_queues=NQ,
    )

    # ---- DRAM tensors (per-core inputs; same program on all 8 cores) ----
    dt_in = lambda n, s, d=F32: nc.dram_tensor(n, s, d, kind="ExternalInput")
    rv = dt_in("rv", [NROW, CIN])       # V layout     [6400, 64]
    rx = dt_in("rx", [NROW, CIN])       # V(x+1)
    ry = dt_in("ry", [NROW, CIN])       # V(y+1)
    rxy = dt_in("rxy", [NROW, CIN])     # V(x+1,y+1)
    offn = dt_in("offn", [128, NG, K, 2])   # offsets (dy,dx), partition-major
    cyxn = dt_in("cyxn", [128, NG, K, 2])   # (iy+ky+15, ix+kx+15)
    # wrapped-16 layout for dma_gather idxs: [j, r, k, s, .] is for pixel
    # i = r*1024 + s*16 + (j%16), tap k (replicated across j//16)
    off16 = dt_in("off16", [128, NR, K, SLOT, 2])
    cyx16 = dt_in("cyx16", [128, NR, K, SLOT, 2])
    w2t = dt_in("w2t", [KC, COUT])      # weight[(k,c), o]
    biasv = dt_in("biasv", [COUT, 1])
    tblD = nc.dram_tensor("tblD", [NROW, ELEM], BF16, kind="Internal")
    # range-0's private copy of slices 0-1: r0 gathers read tblA, so the
    # slice-2/3 stores to tblD have no prior gather readers and issue
    # immediately (tile tracks DRAM hazards at whole-tensor granularity;
    # without this, store-s2 waits for ALL of r0's gather DMAs to drain)
    tblA = nc.dram_tensor("tblA", [2 * RSL, ELEM], BF16, kind="Internal")
    outD = nc.dram_tensor("outD", [COUT, P], F32, kind="ExternalOutput")

    with tile.TileContext(nc) as tc, ExitStack() as ctx:
        consts = ctx.enter_context(tc.tile_pool(name="consts", bufs=1))
        ident = consts.tile([128, 128], BF16)
        make_identity(nc, ident[:])

        # ---- input DMA priority ----
        # sync HWDGE: canvas slices first (table path is gather-critical)
        tsrc = ctx.enter_context(tc.tile_pool(name="tblsrc", bufs=1))
        srcs = {}

        def emit_canvas_load(s):
            rlo = s * RSL
            nrow_s = min(RSL, NROW - rlo)
            APART = nrow_s // 128
            flat = lambda t: t.ap()[rlo:rlo + nrow_s, :].rearrange(
                "(p a) c -> p a c", p=128
            )
            m = s % 2
            v_sb = tsrc.tile([128, APART, CIN], F32, tag=f"v{m}", name=f"v_sb{s}")
            x_sb = tsrc.tile([128, APART, CIN], F32, tag=f"x{m}", name=f"x_sb{s}")
            y_sb = tsrc.tile([128, APART, CIN], F32, tag=f"y{m}", name=f"y_sb{s}")
            xy_sb = tsrc.tile([128, APART, CIN], F32, tag=f"xy{m}",
                              name=f"xy_sb{s}")
            nc.sync.dma_start(v_sb[:], flat(rv))
            nc.sync.dma_start(x_sb[:], flat(rx))
            nc.sync.dma_start(y_sb[:], flat(ry))
            nc.sync.dma_start(xy_sb[:], flat(rxy))
            srcs[s] = (v_sb, x_sb, y_sb, xy_sb, rlo, nrow_s, APART)

        emit_canvas_load(0)
        emit_canvas_load(1)

        # scalar HWDGE: coordinate inputs (r0 first, then wpoly's, then rest)
        wio = ctx.enter_context(tc.tile_pool(name="wio", bufs=1))
        ocs, ccs = [], []
        for r in range(NR):
            oc = wio.tile([128, K, SLOT, 2], F32, tag="oc", name=f"oc{r}")
            cc = wio.tile([128, K, SLOT, 2], F32, tag="cc", name=f"cc{r}")
            nc.scalar.dma_start(oc[:], off16.ap()[:, r])
            nc.scalar.dma_start(cc[:], cyx16.ap()[:, r])
            ocs.append(oc)
            ccs.append(cc)
            if r == 0:
                prept = ctx.enter_context(tc.tile_pool(name="prept", bufs=1))
                offn_sb = prept.tile([128, NG, K, 2], F32, tag="offn")
                cyxn_sb = prept.tile([128, NG, K, 2], F32, tag="cyxn")
                nc.scalar.dma_start(offn_sb[:], offn.ap())
                nc.scalar.dma_start(cyxn_sb[:], cyxn.ap())

        bias_sb = consts.tile([COUT, 1], F32)
        nc.scalar.dma_start(bias_sb[:], biasv.ap())
        wts = []
        wtf = consts.tile([128, NK, COUT], F32, tag="wtf")
        nc.scalar.dma_start(
            wtf[:, 0:4, :], w2t.ap()[0:512, :].rearrange("(j p) o -> p j o", p=128)
        )
        nc.scalar.dma_start(wtf[0:64, 4, :], w2t.ap()[512:KC, :])
        for j in range(NK):
            cs = 128 if j < 4 else 64
            wt = consts.tile([cs, COUT], BF16, tag=f"wt{j}", name=f"wt{j}")
            nc.scalar.copy(wt[:], wtf[0:cs, j, :])
            wts.append(wt)

        prep = ctx.enter_context(tc.tile_pool(name="prep", bufs=1))
        wpoly = prep.tile([128, NG, K, 3], BF16, tag="wpoly")
        idx16 = prep.tile([128, NR, K, SLOT], I16, tag="idx16")

        # ---- DVE emitters (called in a hand-interleaved order below) ----
        wrk = ctx.enter_context(tc.tile_pool(name="wrk", bufs=1))
        tbl_stores = [None] * NSL
        tbl_storesA = [None] * 2

        def emit_wrapped(r):
            """wrapped-16 idx pipeline for range r -> idx16[:, r].
            """
            A = wrk.tile([128, K, SLOT, 2], F32, tag="wA", name=f"wA{r}")
            Bt = wrk.tile([128, K, SLOT, 2], F32, tag="wB", name=f"wB{r}")
            C = wrk.tile([128, K, SLOT, 2], I32, tag="wC", name=f"wC{r}")
            D = wrk.tile([128, K, SLOT, 2], F32, tag="wD", name=f"wD{r}")
            nc.vector.tensor_add(A[:], ocs[r][:], ccs[r][:])
            nc.vector.tensor_scalar(
                Bt[:], A[:], 8.0, 86.99,
                mybir.AluOpType.max, mybir.AluOpType.min,
            )
            if r == 0:
                # keep r0 rows < 80*31+86-648 = 2558 < tblA's 3328 rows
                nc.vector.tensor_scalar(
                    Bt[:, :, :, 0], A[:, :, :, 0], 8.0, 39.99,
                    mybir.AluOpType.max, mybir.AluOpType.min,
                )
            # robust floor: cast, cast back, subtract 1 where the cast went up
            nc.vector.tensor_copy(C[:], Bt[:])
            nc.vector.tensor_copy(D[:], C[:])
            nc.vector.tensor_tensor(A[:], D[:], Bt[:], mybir.AluOpType.is_gt)
            nc.vector.tensor_sub(Bt[:], D[:], A[:])   # floor (y0+16, x0+16)
            # table row = 80*y0 + x0 - 648 (exact small ints in f32)
            nc.vector.scalar_tensor_tensor(
                D[:, :, :, 0], Bt[:, :, :, 0], float(WP), Bt[:, :, :, 1],
                mybir.AluOpType.mult, mybir.AluOpType.add,
            )
            nc.vector.tensor_scalar(
                D[:, :, :, 1], D[:, :, :, 0], -648.0, None, mybir.AluOpType.add,
            )
            nc.vector.tensor_copy(idx16[:, r], D[:, :, :, 1])

        def emit_diff(s):
            """table diff slice s on DVE -> tblD store"""
            v_sb, x_sb, y_sb, xy_sb, rlo, nrow_s, APART = srcs[s]
            m = s % 2
            tbl = wrk.tile([128, APART, 4, CIN], BF16, tag=f"tbl{m}",
                           name=f"tbl{s}")
            t3 = wrk.tile([128, APART, CIN], F32, tag=f"t3{m}", name=f"t3_{s}")
            nc.vector.tensor_copy(tbl[:, :, 0, :], v_sb[:])
            nc.vector.tensor_sub(tbl[:, :, 1, :], y_sb[:], v_sb[:])  # Dy
            nc.vector.tensor_sub(tbl[:, :, 2, :], x_sb[:], v_sb[:])  # Dx
            nc.vector.tensor_sub(t3[:], xy_sb[:], x_sb[:])
            # Dxy = (xy - x) - Dy; reads Dy back in bf16 (2nd-order term,
            # the extra rounding is ~1 ulp of an already-small value)
            nc.vector.tensor_sub(tbl[:, :, 3, :], t3[:], tbl[:, :, 1, :])
            if s < 2:
                tbl_storesA[s] = nc.sync.dma_start(
                    tblA.ap()[rlo:rlo + nrow_s, :].rearrange(
                        "(p a) e -> p a e", p=128
                    ),
                    tbl[:].rearrange("p a v c -> p a (v c)"),
                )
            tbl_stores[s] = nc.sync.dma_start(
                tblD.ap()[rlo:rlo + nrow_s, :].rearrange("(p a) e -> p a e", p=128),
                tbl[:].rearrange("p a v c -> p a (v c)"),
            )

        def emit_wpoly():
            """pixel-major (fy, fx, fy*fx) -> wpoly"""
            A = wrk.tile([128, NG, K, 2], F32, tag="pA", name="pA")
            Bt = wrk.tile([128, NG, K, 2], F32, tag="pB", name="pB")
            C = wrk.tile([128, NG, K, 2], I32, tag="pC", name="pC")
            D = wrk.tile([128, NG, K, 2], F32, tag="pD", name="pD")
            E = wrk.tile([128, NG, K, 2], F32, tag="pE", name="pE")
            nc.vector.tensor_add(A[:], offn_sb[:], cyxn_sb[:])
            nc.vector.tensor_scalar(
                Bt[:], A[:], 8.0, 86.99,
                mybir.AluOpType.max, mybir.AluOpType.min,
            )
            nc.vector.tensor_copy(C[:], Bt[:])
            nc.vector.tensor_copy(D[:], C[:])
            nc.vector.tensor_tensor(E[:], D[:], Bt[:], mybir.AluOpType.is_gt)
            nc.vector.tensor_sub(Bt[:], D[:], E[:])   # floor
            nc.vector.tensor_sub(wpoly[:, :, :, 0:2], A[:], Bt[:])  # (fy, fx)
            nc.vector.tensor_mul(
                wpoly[:, :, :, 2], wpoly[:, :, :, 0], wpoly[:, :, :, 1]
            )  # fy*fx

        # ---- gather + combine + transpose + matmul machinery ----
        colp = ctx.enter_context(tc.tile_pool(name="colp", bufs=2))
        cols = {}
        gpool = ctx.enter_context(tc.tile_pool(name="gath", bufs=8))
        ppool = ctx.enter_context(tc.tile_pool(name="prod", bufs=1))
        apool = ctx.enter_context(tc.tile_pool(name="acc", bufs=2))
        ctp = ctx.enter_context(tc.tile_pool(name="colT", bufs=1))
        cts = []
        for j in range(NK):
            cs = 128 if j < 4 else 64
            cts.append(ctp.tile([cs, P], BF16, tag=f"ct{j}", name=f"ct{j}"))
        wxp = ctx.enter_context(tc.tile_pool(name="wxp", bufs=2))
        pst = ctx.enter_context(tc.tile_pool(name="pst", bufs=4, space="PSUM"))
        psm = ctx.enter_context(tc.tile_pool(name="psm", bufs=2, space="PSUM"))
        obp = ctx.enter_context(tc.tile_pool(name="obp", bufs=2))

        grts = {}
        call_i = [0]

        def emit_gathers(r):
            r0 = r * GRL
            src_t = tblA if r == 0 else tblD
            deps = tbl_storesA if r == 0 else tbl_stores[:SL_NEED[r]]
            for k in range(K):
                grt = gpool.tile([128, GRL, ELEM], BF16, tag="g",
                                 name=f"grt{r}_{k}")
                gi = nc.gpsimd.dma_gather(
                    out_ap=grt[:],
                    in_ap=src_t.ap(),
                    idxs_ap=idx16[:, r, k, :],
                    num_idxs=NIDX,
                    num_idxs_reg=NIDX,
                    elem_size=ELEM,
                    queue_num=call_i[0] % NQ,
                )
                call_i[0] += 1
                if k == 0:
                    # Pool is serial: gate this range's gathers on every
                    # table slice they may touch.
                    for st in deps:
                        add_dep_helper(
                            gi.ins, st.ins,
                            reason="gather reads diff table slice",
                        )
                grts[(r, k)] = grt

        def emit_combine(r, klo=0, khi=K):
            r0 = r * GRL
            if klo == 0:
                col = colp.tile([128, GRL, K, CIN], BF16, tag="col",
                                name=f"col{r}")
                cols[r] = col
            col = cols[r]
            for k in range(klo, khi):
                gv = grts[(r, k)][:].rearrange("p n (v c) -> p n v c", v=4)
                wk = wpoly[:, r0:r0 + GRL, k, :]
                wkb = bass.AP(wk.tensor, wk.offset, list(wk.ap) + [[0, CIN]])
                # expand weights across channels on the (idle) ACT engine:
                # the DVE product then has all-packed 2-byte APs -> 2x mode
                wexp = wxp.tile([128, GRL, 3, CIN], BF16, tag="wexp",
                                name="wexp")
                nc.scalar.copy(wexp[:], wkb)
                pr = ppool.tile([128, GRL, 3, CIN], BF16, tag="pr", name="pr")
                nc.vector.tensor_mul(pr[:], gv[:, :, 1:4, :], wexp[:])
                a1 = apool.tile([128, GRL, CIN], BF16, tag="a1", name="a1")
                a2 = apool.tile([128, GRL, CIN], BF16, tag="a2", name="a2")
                nc.vector.tensor_add(a1[:], gv[:, :, 0, :], pr[:, :, 0, :])
                nc.vector.tensor_add(a2[:], a1[:], pr[:, :, 1, :])
                nc.vector.tensor_add(col[:, :, k, :], a2[:], pr[:, :, 2, :])

        def emit_pe(r):
            r0 = r * GRL
            col = cols[r]
            for half in range(2):
                for g_i in range(r0 + 4 * half, r0 + 4 * half + 4):
                    for j in range(NK):
                        cs = 128 if j < 4 else 64
                        src = bass.AP(
                            col[:].tensor,
                            col[:].offset + (g_i - r0) * (K * CIN) + j * 128,
                            [list(col[:].ap[0]), [1, cs]],
                        )
                        ptile = pst.tile([cs, 128], BF16, tag="pt", name="pt")
                        nc.tensor.transpose(ptile[:], src, ident[:])
                        nc.scalar.copy(
                            cts[j][:, g_i * 128:(g_i + 1) * 128], ptile[:]
                        )
                n = r0 // 4 + half
                pm = psm.tile([COUT, P // NCH], F32, tag="pm", name="pm")
                for j in range(NK):
                    nc.tensor.matmul(
                        pm[:],
                        wts[j][:],
                        cts[j][:, n * (P // NCH):(n + 1) * (P // NCH)],
                        start=(j == 0),
                        stop=(j == NK - 1),
                    )
                ob = obp.tile([COUT, P // NCH], F32, tag="ob", name="ob")
                nc.scalar.activation(
                    ob[:], pm[:], mybir.ActivationFunctionType.Identity,
                    bias=bias_sb[:], scale=1.0,
                )
                nc.sync.dma_start(
                    outD.ap()[:, n * (P // NCH):(n + 1) * (P // NCH)], ob[:]
                )

        # ---- hand-interleaved emission: keeps DVE feeding the gather
        # stream (idx + table slices first, combines started early enough
        # to free the gather tile ring before the next range needs it) ----
        emit_wrapped(0)
        emit_diff(0)
        emit_diff(1)
        emit_canvas_load(2)
        emit_canvas_load(3)
        emit_gathers(0)
        emit_wpoly()
        emit_wrapped(1)
        emit_diff(2)
        emit_diff(3)
        emit_gathers(1)
        emit_combine(0)
        emit_wrapped(2)
        emit_wrapped(3)
        emit_gathers(2)
        emit_pe(0)
        emit_combine(1)
        emit_gathers(3)
        emit_pe(1)
        emit_combine(2)
        # combine(3) BEFORE pe(2): its 9 ACT weight-expansion copies queue
        # ahead of pe(2)'s 40 cts copies on the ACT engine, so the tail
        # combine isn't serialized behind them (combine3 depends on nothing
        # from pe2; pe2's outputs feed only its own non-critical out-chunks)
        emit_combine(3)
        emit_pe(2)
        emit_pe(3)

    nc.compile()
    return nc


def _host_prep(rgb, offsets, weight, bias):
    """Layout-only host prep -> per-core input maps."""
    rgb = np.ascontiguousarray(np.asarray(rgb, dtype=np.float32))
    offsets = np.ascontiguousarray(np.asarray(offsets, dtype=np.float32))
    weight = np.asarray(weight, dtype=np.float32)
    bias = np.asarray(bias, dtype=np.float32)

    w2t = np.ascontiguousarray(
        weight.transpose(2, 3, 1, 0).reshape(KC, COUT)
    )
    biasv = np.ascontiguousarray(bias.reshape(COUT, 1))

    ky = (np.arange(K) // 3).astype(np.float32)
    kx = (np.arange(K) % 3).astype(np.float32)
    pix = np.arange(P)
    iy = (pix // W).astype(np.float32)
    ix = (pix % W).astype(np.float32)

    # natural layout [128, 32, 9, 2]: pixel p=(g*128+part), (y, x) pairs
    def nat(base, kk):
        c = base[:, None] + kk[None, :]          # [4096, 9]
        return c.reshape(NG, 128, K).transpose(1, 0, 2)

    cyxn = np.ascontiguousarray(
        np.stack([nat(iy + 15.0, ky), nat(ix + 15.0, kx)], axis=-1)
    )

    # wrapped-16 layout [128, NR, K, SLOT, 2]: [j, r, k, s] is pixel
    # i = r*1024 + s*16 + (j%16), tap k
    jj = np.arange(128) % 16                      # [128]
    ss = np.arange(SLOT)                          # [64]
    rr = np.arange(NR)                            # [4]
    i16 = (rr[None, :, None] * 1024
           + ss[None, None, :] * 16
           + jj[:, None, None])                   # [128, NR, SLOT]
    iy16 = (i16 // W).astype(np.float32)
    ix16 = (i16 % W).astype(np.float32)
    cyx16 = np.ascontiguousarray(np.stack([
        iy16[:, :, None, :] + ky[None, None, :, None] + 15.0,
        ix16[:, :, None, :] + kx[None, None, :, None] + 15.0,
    ], axis=-1))                                  # [128, NR, K, SLOT, 2]

    in_maps = []
    for b in range(B):
        canvas = np.zeros((CIN, H + 18, W + 18), np.float32)
        canvas[:, PAD:PAD + H, PAD:PAD + W] = rgb[b]
        mk = lambda sy, sx: np.ascontiguousarray(
            canvas[:, sy:sy + HP, sx:sx + WP].transpose(1, 2, 0).reshape(NROW, CIN)
        )
        offs = offsets[b].reshape(2 * K, P)
        o = offs.reshape(K, 2, P)                 # [k, (dy,dx), pixel]
        offn_pm = np.ascontiguousarray(
            offs.T.reshape(NG, 128, K, 2).transpose(1, 0, 2, 3)
        )                                         # [128, 32, 9, 2]
        dy16 = o[:, 0, :][:, i16]                 # [k, 128, NR, SLOT]
        dx16 = o[:, 1, :][:, i16]
        off16 = np.ascontiguousarray(
            np.stack([dy16, dx16], axis=-1).transpose(1, 2, 0, 3, 4)
        )                                         # [128, NR, K, SLOT, 2]
        in_maps.append({
            "rv": mk(0, 0), "rx": mk(0, 1), "ry": mk(1, 0), "rxy": mk(1, 1),
            "offn": offn_pm, "cyxn": cyxn,
            "off16": off16, "cyx16": cyx16,
            "w2t": w2t, "biasv": biasv,
        })
    return in_maps


def _axon_reset():
    try:
        import ctypes

        import jax

        jax.devices()
        lib = ctypes.CDLL("/opt/axon/libaxon_pjrt.so")
        lib.axon_reset.restype = ctypes.c_int64
        lib.axon_reset()
    except Exception:
        pass


def kernel(rgb, offsets, weight, bias):
    global _PROG, LAST_EXEC_NS
    if _PROG is None:
        _PROG = _build_program()
    in_maps = _host_prep(rgb, offsets, weight, bias)
    try:
        res = run_bass_kernel_spmd(
            _PROG, in_maps, core_ids=list(range(B)), trace=TRACE
        )
    except Exception:
        # a previous crashed run can leave the device wedged; reset + retry
        _axon_reset()
        res = run_bass_kernel_spmd(
            _PROG, in_maps, core_ids=list(range(B)), trace=TRACE
        )
    LAST_EXEC_NS = res.exec_time_ns
    out = np.stack([res.results[b]["outD"] for b in range(B)])
    return out.reshape(B, COUT, H, W).astype(np.float32)


# revision 31
# speedup vs baseline: 1.0844x; 1.0844x over previous
"""Trainium2 Bass kernel for deformable conv2d (nn_DeformByDepthConv2d).

Strategy: data-parallel over batch (8 images -> 8 NeuronCores). Per core:
  1. Build a bilinear "difference table" in DRAM: for every padded pixel
     position r=(y,x), row = [V, Dy, Dx, Dxy] (64 ch each, bf16, 512B):
       V   = rgb[:, y, x]
       Dy  = rgb[:, y+1, x] - rgb[:, y, x]
       Dx  = rgb[:, y, x+1] - rgb[:, y, x]
       Dxy = rgb[:, y+1, x+1] - rgb[:, y+1, x] - rgb[:, y, x+1] + rgb[:, y, x]
     The bilinear sample is then exactly: V + fy*Dy + fx*Dx + fx*fy*Dxy
     -- so ONE gathered row per (tap, pixel) fetches everything needed.
  2. Compute gather rows + poly weights (fy, fx, fx*fy) from `offsets`
     on-device (DVE). Two coordinate pipelines: pixel-major for wpoly,
     and the dma_gather wrapped-16 idx layout (int16) directly from
     host-rearranged (layout-only) coordinate inputs. y and x are
     processed interleaved (.., 2) to halve instruction count.
  3. Gather via batched ext-isa dma_gather (InstDMAGatherAnt): 36 calls
     of 1024 rows x 512B, round-robined over 4 SWDGE queues so the 4x16
     DMA rings drain in parallel (~150 GB/s vs 46 GB/s on one queue).
     Calls are paced by a bufs=8 tile ring (the DVE combine consuming a
     gather tile gates the call 8 slots later) -- this runtime has no
     ring-overflow reclaim, so outstanding descriptors must stay under
     ~2 calls/queue. num_idxs > 1024 per call crashes the runtime.
     Pool runs ONLY the gathers; canvas loads go first on the sync
     HWDGE queue and coordinate inputs on the scalar HWDGE queue so the
     first gather can start ~25us in (was 111us).
  4. DVE poly combine -> col[128px, 32, tap, 64ch] (bf16).
  5. PE transposes col -> colT[(tap,ch) 576, 4096px], then the conv is a
     [576,128]^T x [576,4096] matmul accumulated over 5 K-chunks in PSUM.
  6. ACT adds bias on PSUM->SBUF copy; DMA out fp32 [128, 4096].

Host side does layout-only prep (transposes/padding/replication of inputs);
all arithmetic (diffs, coords, weights, conv) runs on device.
"""

import sys

sys.path.insert(0, "/opt/trn_rl_repo")

from contextlib import ExitStack

import numpy as np

import concourse.bass as bass
import concourse.tile as tile
from concourse import bacc, mybir
from concourse.bass_utils import run_bass_kernel_spmd
from concourse.masks import make_identity
from concourse.tile import add_dep_helper

F32 = mybir.dt.float32
BF16 = mybir.dt.bfloat16
I16 = mybir.dt.int16
I32 = mybir.dt.int32

B, CIN, COUT, KH, KW = 8, 64, 128, 3, 3
H = W = 64
K = KH * KW            # 9 taps
P = H * W              # 4096 pixels
PAD = 8                # table padding on each side
HP = WP = H + 2 * PAD  # 80
NROW = HP * WP         # 6400 table rows
NG = P // 128          # 32 pixel groups of 128
ELEM = 4 * CIN         # table row: V|Dy|Dx|Dxy x 64ch = 256 bf16 = 512B
KC = K * CIN           # 576 contraction rows
NK = 5                 # K chunks: 4x128 + 1x64
NCH = 8                # output N chunks of 512

NQ = 4                 # SWDGE queues (max supported)
NR = 4                 # gather ranges
GRL = NG // NR         # pixel groups per range = 8
NIDX = GRL * 128       # rows per dma_gather call = 1024 (hard runtime max)
SLOT = NIDX // 16      # idx free slots per call = 64

NSL = 4                # table slices
RSL = 1664             # rows per slice; last = 1408
# per-range gather view (rows) and matching y-clamp hi (y+16 scale):
# range r only reads rows < VIEW[r] because its clamped y0 <= CLAMP_HI[r]
VIEW = [2560, 3840, 5120, 6400]
CLAMP_HI = [39.99, 55.99, 71.99, 86.99]
# table slices each range's gathers must wait for
SL_NEED = [2, 3, 4, 4]

TRACE = False
LAST_EXEC_NS = None
_PROG = None


def _build_program():
    nc = bacc.Bacc(
        "TRN2", target_bir_lowering=False, debug=False, num_devices=8,
        num_swdge_queues=NQ,
    )

    # ---- DRAM tensors (per-core inputs; same program on all 8 cores) ----
    dt_in = lambda n, s, d=F32: nc.dram_tensor(n, s, d, kind="ExternalInput")
    rv = dt_in("rv", [NROW, CIN])       # V layout     [6400, 64]
    rx = dt_in("rx", [NROW, CIN])       # V(x+1)
    ry = dt_in("ry", [NROW, CIN])       # V(y+1)
    rxy = dt_in("rxy", [NROW, CIN])     # V(x+1,y+1)
    offn = dt_in("offn", [128, NG, K, 2])   # offsets (dy,dx), partition-major
    cyxn = dt_in("cyxn", [128, NG, K, 2])   # (iy+ky+15, ix+kx+15)
    # wrapped-16 layout for dma_gather idxs: [j, r, k, s, .] is for pixel
    # i = r*1024 + s*16 + (j%16), tap k (replicated across j//16)
    off16 = dt_in("off16", [128, NR, K, SLOT, 2])
    cyx16 = dt_in("cyx16", [128, NR, K, SLOT, 2])
    w2t = dt_in("w2t", [KC, COUT])      # weight[(k,c), o]
    biasv = dt_in("biasv", [COUT, 1])
    tblD = nc.dram_tensor("tblD", [NROW, ELEM], BF16, kind="Internal")
    outD = nc.dram_tensor("outD", [COUT, P], F32, kind="ExternalOutput")

    with tile.TileContext(nc) as tc, ExitStack() as ctx:
        consts = ctx.enter_context(tc.tile_pool(name="consts", bufs=1))
        ident = consts.tile([128, 128], BF16)
        make_identity(nc, ident[:])

        # ---- input DMA priority ----
        # sync HWDGE: canvas slices first (table path is gather-critical)
        tsrc = ctx.enter_context(tc.tile_pool(name="tblsrc", bufs=1))
        srcs = {}

        def emit_canvas_load(s):
            rlo = s * RSL
            nrow_s = min(RSL, NROW - rlo)
            APART = nrow_s // 128
            flat = lambda t: t.ap()[rlo:rlo + nrow_s, :].rearrange(
                "(p a) c -> p a c", p=128
            )
            m = s % 2
            v_sb = tsrc.tile([128, APART, CIN], F32, tag=f"v{m}", name=f"v_sb{s}")
            x_sb = tsrc.tile([128, APART, CIN], F32, tag=f"x{m}", name=f"x_sb{s}")
            y_sb = tsrc.tile([128, APART, CIN], F32, tag=f"y{m}", name=f"y_sb{s}")
            xy_sb = tsrc.tile([128, APART, CIN], F32, tag=f"xy{m}",
                              name=f"xy_sb{s}")
            nc.sync.dma_start(v_sb[:], flat(rv))
            nc.sync.dma_start(x_sb[:], flat(rx))
            nc.sync.dma_start(y_sb[:], flat(ry))
            nc.sync.dma_start(xy_sb[:], flat(rxy))
            srcs[s] = (v_sb, x_sb, y_sb, xy_sb, rlo, nrow_s, APART)

        # r0 coordinate chunks at the FRONT of the sync queue: they gate
        # the DVE wrapped-idx pipeline that the whole head chain hangs off
        # (on the scalar queue they landed ~15us in, behind the act-table
        # load; here ~7us)
        wio = ctx.enter_context(tc.tile_pool(name="wio", bufs=1))
        ocs, ccs = [], []
        oc0 = wio.tile([128, K, SLOT, 2], F32, tag="oc", name="oc0")
        cc0 = wio.tile([128, K, SLOT, 2], F32, tag="cc", name="cc0")
        nc.sync.dma_start(oc0[:], off16.ap()[:, 0])
        nc.sync.dma_start(cc0[:], cyx16.ap()[:, 0])
        ocs.append(oc0)
        ccs.append(cc0)

        emit_canvas_load(0)
        emit_canvas_load(1)

        # scalar HWDGE: remaining coordinate inputs
        prept = ctx.enter_context(tc.tile_pool(name="prept", bufs=1))
        offn_sb = prept.tile([128, NG, K, 2], F32, tag="offn")
        cyxn_sb = prept.tile([128, NG, K, 2], F32, tag="cyxn")
        nc.scalar.dma_start(offn_sb[:], offn.ap())
        nc.scalar.dma_start(cyxn_sb[:], cyxn.ap())
        for r in range(1, NR):
            oc = wio.tile([128, K, SLOT, 2], F32, tag="oc", name=f"oc{r}")
            cc = wio.tile([128, K, SLOT, 2], F32, tag="cc", name=f"cc{r}")
            nc.scalar.dma_start(oc[:], off16.ap()[:, r])
            nc.scalar.dma_start(cc[:], cyx16.ap()[:, r])
            ocs.append(oc)
            ccs.append(cc)

        bias_sb = consts.tile([COUT, 1], F32)
        nc.scalar.dma_start(bias_sb[:], biasv.ap())
        wts = []
        wtf = consts.tile([128, NK, COUT], F32, tag="wtf")
        nc.scalar.dma_start(
            wtf[:, 0:4, :], w2t.ap()[0:512, :].rearrange("(j p) o -> p j o", p=128)
        )
        nc.scalar.dma_start(wtf[0:64, 4, :], w2t.ap()[512:KC, :])
        for j in range(NK):
            cs = 128 if j < 4 else 64
            wt = consts.tile([cs, COUT], BF16, tag=f"wt{j}", name=f"wt{j}")
            nc.scalar.copy(wt[:], wtf[0:cs, j, :])
            wts.append(wt)

        prep = ctx.enter_context(tc.tile_pool(name="prep", bufs=1))
        wpoly = prep.tile([128, NG, K, 3], BF16, tag="wpoly")
        idx16 = prep.tile([128, NR, K, SLOT], I16, tag="idx16")

        # ---- DVE emitters (called in a hand-interleaved order below) ----
        wrk = ctx.enter_context(tc.tile_pool(name="wrk", bufs=1))
        tbl_stores = [None] * NSL

        def emit_wrapped(r):
            """wrapped-16 idx pipeline for range r -> idx16[:, r].
            """
            A = wrk.tile([128, K, SLOT, 2], F32, tag="wA", name=f"wA{r}")
            Bt = wrk.tile([128, K, SLOT, 2], F32, tag="wB", name=f"wB{r}")
            C = wrk.tile([128, K, SLOT, 2], I32, tag="wC", name=f"wC{r}")
            D = wrk.tile([128, K, SLOT, 2], F32, tag="wD", name=f"wD{r}")
            nc.vector.tensor_add(A[:], ocs[r][:], ccs[r][:])
            nc.vector.tensor_scalar(
                Bt[:], A[:], 8.0, 86.99,
                mybir.AluOpType.max, mybir.AluOpType.min,
            )
            # robust floor: cast, cast back, subtract 1 where the cast went up
            nc.vector.tensor_copy(C[:], Bt[:])
            nc.vector.tensor_copy(D[:], C[:])
            nc.vector.tensor_tensor(A[:], D[:], Bt[:], mybir.AluOpType.is_gt)
            nc.vector.tensor_sub(Bt[:], D[:], A[:])   # floor (y0+16, x0+16)
            # table row = 80*y0 + x0 - 648 (exact small ints in f32)
            nc.vector.scalar_tensor_tensor(
                D[:, :, :, 0], Bt[:, :, :, 0], float(WP), Bt[:, :, :, 1],
                mybir.AluOpType.mult, mybir.AluOpType.add,
            )
            nc.vector.tensor_scalar(
                D[:, :, :, 1], D[:, :, :, 0], -648.0, None, mybir.AluOpType.add,
            )
            nc.vector.tensor_copy(idx16[:, r], D[:, :, :, 1])

        def emit_diff(s):
            """table diff slice s on DVE -> tblD store"""
            v_sb, x_sb, y_sb, xy_sb, rlo, nrow_s, APART = srcs[s]
            m = s % 2
            tbl = wrk.tile([128, APART, 4, CIN], BF16, tag=f"tbl{m}",
                           name=f"tbl{s}")
            t3 = wrk.tile([128, APART, CIN], F32, tag=f"t3{m}", name=f"t3_{s}")
            nc.vector.tensor_copy(tbl[:, :, 0, :], v_sb[:])
            nc.vector.tensor_sub(tbl[:, :, 1, :], y_sb[:], v_sb[:])  # Dy
            nc.vector.tensor_sub(tbl[:, :, 2, :], x_sb[:], v_sb[:])  # Dx
            nc.vector.tensor_sub(t3[:], xy_sb[:], x_sb[:])
            # Dxy = (xy - x) - Dy; reads Dy back in bf16 (2nd-order term,
            # the extra rounding is ~1 ulp of an already-small value)
            nc.vector.tensor_sub(tbl[:, :, 3, :], t3[:], tbl[:, :, 1, :])
            tbl_stores[s] = nc.sync.dma_start(
                tblD.ap()[rlo:rlo + nrow_s, :].rearrange("(p a) e -> p a e", p=128),
                tbl[:].rearrange("p a v c -> p a (v c)"),
            )

        def emit_wpoly():
            """pixel-major (fy, fx, fy*fx) -> wpoly"""
            A = wrk.tile([128, NG, K, 2], F32, tag="pA", name="pA")
            Bt = wrk.tile([128, NG, K, 2], F32, tag="pB", name="pB")
            C = wrk.tile([128, NG, K, 2], I32, tag="pC", name="pC")
            D = wrk.tile([128, NG, K, 2], F32, tag="pD", name="pD")
            E = wrk.tile([128, NG, K, 2], F32, tag="pE", name="pE")
            nc.vector.tensor_add(A[:], offn_sb[:], cyxn_sb[:])
            nc.vector.tensor_scalar(
                Bt[:], A[:], 8.0, 86.99,
                mybir.AluOpType.max, mybir.AluOpType.min,
            )
            nc.vector.tensor_copy(C[:], Bt[:])
            nc.vector.tensor_copy(D[:], C[:])
            nc.vector.tensor_tensor(E[:], D[:], Bt[:], mybir.AluOpType.is_gt)
            nc.vector.tensor_sub(Bt[:], D[:], E[:])   # floor
            nc.vector.tensor_sub(wpoly[:, :, :, 0:2], A[:], Bt[:])  # (fy, fx)
            nc.vector.tensor_mul(
                wpoly[:, :, :, 2], wpoly[:, :, :, 0], wpoly[:, :, :, 1]
            )  # fy*fx

        # ---- gather + combine + transpose + matmul machinery ----
        colp = ctx.enter_context(tc.tile_pool(name="colp", bufs=2))
        cols = {}
        gpool = ctx.enter_context(tc.tile_pool(name="gath", bufs=8))
        ppool = ctx.enter_context(tc.tile_pool(name="prod", bufs=1))
        apool = ctx.enter_context(tc.tile_pool(name="acc", bufs=2))
        ctp = ctx.enter_context(tc.tile_pool(name="colT", bufs=1))
        cts = []
        for j in range(NK):
            cs = 128 if j < 4 else 64
            cts.append(ctp.tile([cs, P], BF16, tag=f"ct{j}", name=f"ct{j}"))
        wxp = ctx.enter_context(tc.tile_pool(name="wxp", bufs=2))
        pst = ctx.enter_context(tc.tile_pool(name="pst", bufs=4, space="PSUM"))
        psm = ctx.enter_context(tc.tile_pool(name="psm", bufs=2, space="PSUM"))
        obp = ctx.enter_context(tc.tile_pool(name="obp", bufs=2))

        grts = {}
        call_i = [0]

        def emit_gathers(r):
            r0 = r * GRL
            for k in range(K):
                grt = gpool.tile([128, GRL, ELEM], BF16, tag="g",
                                 name=f"grt{r}_{k}")
                gi = nc.gpsimd.dma_gather(
                    out_ap=grt[:],
                    in_ap=tblD.ap(),
                    idxs_ap=idx16[:, r, k, :],
                    num_idxs=NIDX,
                    num_idxs_reg=NIDX,
                    elem_size=ELEM,
                    queue_num=call_i[0] % NQ,
                )
                call_i[0] += 1
                if k == 0:
                    # Pool is serial: gate this range's gathers on every
                    # table slice they may touch.
                    for s in range(SL_NEED[r]):
                        add_dep_helper(
                            gi.ins, tbl_stores[s].ins,
                            reason="gather reads diff table slice",
                        )
                grts[(r, k)] = grt

        def emit_combine(r, klo=0, khi=K):
            r0 = r * GRL
            if klo == 0:
                col = colp.tile([128, GRL, K, CIN], BF16, tag="col",
                                name=f"col{r}")
                cols[r] = col
            col = cols[r]
            for k in range(klo, khi):
                gv = grts[(r, k)][:].rearrange("p n (v c) -> p n v c", v=4)
                wk = wpoly[:, r0:r0 + GRL, k, :]
                wkb = bass.AP(wk.tensor, wk.offset, list(wk.ap) + [[0, CIN]])
                # expand weights across channels on the (idle) ACT engine:
                # the DVE product then has all-packed 2-byte APs -> 2x mode
                wexp = wxp.tile([128, GRL, 3, CIN], BF16, tag="wexp",
                                name="wexp")
                nc.scalar.copy(wexp[:], wkb)
                pr = ppool.tile([128, GRL, 3, CIN], BF16, tag="pr", name="pr")
                nc.vector.tensor_mul(pr[:], gv[:, :, 1:4, :], wexp[:])
                a1 = apool.tile([128, GRL, CIN], BF16, tag="a1", name="a1")
                a2 = apool.tile([128, GRL, CIN], BF16, tag="a2", name="a2")
                nc.vector.tensor_add(a1[:], gv[:, :, 0, :], pr[:, :, 0, :])
                nc.vector.tensor_add(a2[:], a1[:], pr[:, :, 1, :])
                nc.vector.tensor_add(col[:, :, k, :], a2[:], pr[:, :, 2, :])

        def emit_pe(r):
            r0 = r * GRL
            col = cols[r]
            for half in range(2):
                for g_i in range(r0 + 4 * half, r0 + 4 * half + 4):
                    for j in range(NK):
                        cs = 128 if j < 4 else 64
                        src = bass.AP(
                            col[:].tensor,
                            col[:].offset + (g_i - r0) * (K * CIN) + j * 128,
                            [list(col[:].ap[0]), [1, cs]],
                        )
                        ptile = pst.tile([cs, 128], BF16, tag="pt", name="pt")
                        nc.tensor.transpose(ptile[:], src, ident[:])
                        nc.scalar.copy(
                            cts[j][:, g_i * 128:(g_i + 1) * 128], ptile[:]
                        )
                n = r0 // 4 + half
                pm = psm.tile([COUT, P // NCH], F32, tag="pm", name="pm")
                for j in range(NK):
                    nc.tensor.matmul(
                        pm[:],
                        wts[j][:],
                        cts[j][:, n * (P // NCH):(n + 1) * (P // NCH)],
                        start=(j == 0),
                        stop=(j == NK - 1),
                    )
                ob = obp.tile([COUT, P // NCH], F32, tag="ob", name="ob")
                nc.scalar.activation(
                    ob[:], pm[:], mybir.ActivationFunctionType.Identity,
                    bias=bias_sb[:], scale=1.0,
                )
                nc.sync.dma_start(
                    outD.ap()[:, n * (P // NCH):(n + 1) * (P // NCH)], ob[:]
                )

        # ---- hand-interleaved emission: keeps DVE feeding the gather
        # stream (idx + table slices first, combines started early enough
        # to free the gather tile ring before the next range needs it) ----
        emit_wrapped(0)
        emit_diff(0)
        emit_diff(1)
        emit_canvas_load(2)
        emit_canvas_load(3)
        emit_gathers(0)
        emit_wpoly()
        emit_wrapped(1)
        emit_diff(2)
        emit_diff(3)
        emit_gathers(1)
        emit_combine(0)
        emit_wrapped(2)
        emit_wrapped(3)
        emit_gathers(2)
        emit_pe(0)
        emit_combine(1)
        emit_gathers(3)
        emit_pe(1)
        emit_combine(2)
        # combine(3) BEFORE pe(2): its 9 ACT weight-expansion copies queue
        # ahead of pe(2)'s 40 cts copies on the ACT engine, so the tail
        # combine isn't serialized behind them (combine3 depends on nothing
        # from pe2; pe2's outputs feed only its own non-critical out-chunks)
        emit_combine(3)
        emit_pe(2)
        emit_pe(3)

    nc.compile()
    return nc


def _host_prep(rgb, offsets, weight, bias):
    """Layout-only host prep -> per-core input maps."""
    rgb = np.ascontiguousarray(np.asarray(rgb, dtype=np.float32))
    offsets = np.ascontiguousarray(np.asarray(offsets, dtype=np.float32))
    weight = np.asarray(weight, dtype=np.float32)
    bias = np.asarray(bias, dtype=np.float32)

    w2t = np.ascontiguousarray(
        weight.transpose(2, 3, 1, 0).reshape(KC, COUT)
    )
    biasv = np.ascontiguousarray(bias.reshape(COUT, 1))

    ky = (np.arange(K) // 3).astype(np.float32)
    kx = (np.arange(K) % 3).astype(np.float32)
    pix = np.arange(P)
    iy = (pix // W).astype(np.float32)
    ix = (pix % W).astype(np.float32)

    # natural layout [128, 32, 9, 2]: pixel p=(g*128+part), (y, x) pairs
    def nat(base, kk):
        c = base[:, None] + kk[None, :]          # [4096, 9]
        return c.reshape(NG, 128, K).transpose(1, 0, 2)

    cyxn = np.ascontiguousarray(
        np.stack([nat(iy + 15.0, ky), nat(ix + 15.0, kx)], axis=-1)
    )

    # wrapped-16 layout [128, NR, K, SLOT, 2]: [j, r, k, s] is pixel
    # i = r*1024 + s*16 + (j%16), tap k
    jj = np.arange(128) % 16                      # [128]
    ss = np.arange(SLOT)                          # [64]
    rr = np.arange(NR)                            # [4]
    i16 = (rr[None, :, None] * 1024
           + ss[None, None, :] * 16
           + jj[:, None, None])                   # [128, NR, SLOT]
    iy16 = (i16 // W).astype(np.float32)
    ix16 = (i16 % W).astype(np.float32)
    cyx16 = np.ascontiguousarray(np.stack([
        iy16[:, :, None, :] + ky[None, None, :, None] + 15.0,
        ix16[:, :, None, :] + kx[None, None, :, None] + 15.0,
    ], axis=-1))                                  # [128, NR, K, SLOT, 2]

    in_maps = []
    for b in range(B):
        canvas = np.zeros((CIN, H + 18, W + 18), np.float32)
        canvas[:, PAD:PAD + H, PAD:PAD + W] = rgb[b]
        mk = lambda sy, sx: np.ascontiguousarray(
            canvas[:, sy:sy + HP, sx:sx + WP].transpose(1, 2, 0).reshape(NROW, CIN)
        )
        offs = offsets[b].reshape(2 * K, P)
        o = offs.reshape(K, 2, P)                 # [k, (dy,dx), pixel]
        offn_pm = np.ascontiguousarray(
            offs.T.reshape(NG, 128, K, 2).transpose(1, 0, 2, 3)
        )                                         # [128, 32, 9, 2]
        dy16 = o[:, 0, :][:, i16]                 # [k, 128, NR, SLOT]
        dx16 = o[:, 1, :][:, i16]
        off16 = np.ascontiguousarray(
            np.stack([dy16, dx16], axis=-1).transpose(1, 2, 0, 3, 4)
        )                                         # [128, NR, K, SLOT, 2]
        in_maps.append({
            "rv": mk(0, 0), "rx": mk(0, 1), "ry": mk(1, 0), "rxy": mk(1, 1),
            "offn": offn_pm, "cyxn": cyxn,
            "off16": off16, "cyx16": cyx16,
            "w2t": w2t, "biasv": biasv,
        })
    return in_maps


def _axon_reset():
    try:
        import ctypes

        import jax

        jax.devices()
        lib = ctypes.CDLL("/opt/axon/libaxon_pjrt.so")
        lib.axon_reset.restype = ctypes.c_int64
        lib.axon_reset()
    except Exception:
        pass


def kernel(rgb, offsets, weight, bias):
    global _PROG, LAST_EXEC_NS
    if _PROG is None:
        _PROG = _build_program()
    in_maps = _host_prep(rgb, offsets, weight, bias)
    try:
        res = run_bass_kernel_spmd(
            _PROG, in_maps, core_ids=list(range(B)), trace=TRACE
        )
    except Exception:
        # a previous crashed run can leave the device wedged; reset + retry
        _axon_reset()
        res = run_bass_kernel_spmd(
            _PROG, in_maps, core_ids=list(range(B)), trace=TRACE
        )
    LAST_EXEC_NS = res.exec_time_ns
    out = np.stack([res.results[b]["outD"] for b in range(B)])
    return out.reshape(B, COUT, H, W).astype(np.float32)
